# revision 12
# baseline (speedup 1.0000x reference)
"""Trainium2 Bass kernel for nn_ClassicalSelfAttention (B=4, S=2048, E=1024).

Reference computation (fp32):
    w_qkv = rotation_params.reshape(3E, E); w_out = entangle_params.reshape(E, E)
    qkv = x @ w_qkv.T; q, k, v = split(qkv)
    scores = (q / sqrt(64)) @ k.T          # full-E attention, no heads
    attn = softmax(scores, axis=-1)
    out = (attn @ v) @ w_out.T
    result = sigmoid(out @ gate_w.T) * out

Host-side weight fusion (pure algebra, done once on CPU):
    scores = q @ k.T = x @ (wq.T @ wk) @ x.T          -> M  = wq.T @ wk
    (attn @ v) @ w_out.T = attn @ (x @ (w_out@wv).T)  -> W2 = w_out @ wv
so the k-projection and out-projection matmuls disappear from the device.

Sharding: 8 cores = 4 batches x 2 query-halves. Each core computes vo for its
whole batch (duplicated within the pair) and attention + gate for its 1024
queries. Key order is rotated per query-half so each core's queries are
always columns 0:1024 of its (host-pre-transposed) x^T input — softmax and
attn@vo are permutation-invariant in key order.

Precision: all matmul operands are bf16 (softmax statistics, PSUM
accumulation, and the final gate/multiply epilogue stay fp32). bf16 halves
DMA traffic, lets the whole working set stay resident in SBUF (no DRAM
spill), and its ~97ns LDWEIGHTS hides completely under 512-col matmuls so
every phase runs at the PE issue limit. Measured rel err ~7e-3 (gate 2e-2).

Layout (feature-major throughout):
    xT [e, s]  (resident through 2a)
    vo [s, f] = xT-stat @ W2T           (vo = x @ W2.T, computed first)
    yT [f, s] = M-contract @ xq         (y = x @ M)
    scores [qi, kj] = yT-stat @ xT -> softmax along free dim -> attn
    PE-transpose attn -> attnT [kj, qi]
    outT [f, qi] = vo.T-contract @ attnT
    gateT = gw-contract @ outT;  result^T = sigmoid(gateT) * outT
Host untransposes the per-core [E, 1024] result tiles.

Matmul loops interleave 4 PSUM accumulation chains (pairs of output tiles x
free-dim chunks) to hide LDWEIGHTS; each query block's 16 attn transposes
are emitted between the next block's score matmuls for the same reason.
"""

from contextlib import ExitStack

import numpy as np

import concourse.bass as bass
import concourse.tile as tile
from concourse import bacc, mybir
from concourse.bass_utils import run_bass_kernel_spmd
from concourse.masks import make_identity

F32 = mybir.dt.float32
F32R = mybir.dt.float32r
BF16 = mybir.dt.bfloat16

P = 128
E = 1024
B = 4
S = 2048
SK = S            # keys per core (full batch sequence)
SQ = S // 2       # queries per core (half)
ET = E // P       # 8 e-tiles
KT = SK // P      # 16 key tiles
NC = 512          # moving-operand chunk (f32r full speed needs >=256, max 512)
SKC = SK // NC    # 4
SQC = SQ // NC    # 2
FC = E // NC      # 2
NCORES = 8
SCALE = 1.0 / 8.0  # 1/sqrt(head_dim=64), folded into exp()


def _round_fp32r(x: np.ndarray) -> np.ndarray:
    """Round-to-nearest-even to fp32r (11-bit mantissa; low 12 bits zero)."""
    u = np.ascontiguousarray(x, dtype=np.float32).view(np.uint32).astype(np.uint64)
    r = (u + 0x7FF + ((u >> 12) & 1)) & ~np.uint64(0xFFF)
    return r.astype(np.uint32).view(np.float32)


def _build_nc():
    nc = bacc.Bacc("TRN2", target_bir_lowering=False, debug=False,
                   num_devices=NCORES)
    xT = nc.dram_tensor("xT", [E, SK], BF16, kind="ExternalInput").ap()
    mM = nc.dram_tensor("mM", [E, E], BF16, kind="ExternalInput").ap()
    w2T = nc.dram_tensor("w2T", [E, E], BF16, kind="ExternalInput").ap()
    gwT = nc.dram_tensor("gwT", [E, E], BF16, kind="ExternalInput").ap()
    outT = nc.dram_tensor("outT", [E, SQ], F32, kind="ExternalOutput").ap()

    with tile.TileContext(nc) as tc, ExitStack() as ctx:
        _emit(tc, ctx, xT, mM, w2T, gwT, outT)
    nc.compile()
    return nc


def _emit(tc, ctx, xT, mM, w2T, gwT, outT):
    nc = tc.nc
    Exp = mybir.ActivationFunctionType.Exp
    Sigmoid = mybir.ActivationFunctionType.Sigmoid

    singles = ctx.enter_context(tc.tile_pool(name="singles", bufs=1))
    ps_mm = ctx.enter_context(tc.tile_pool(name="ps_mm", bufs=6, space="PSUM"))

    # Long-lived SBUF pools, created longest-lived first (LIFO release):
    # vo lives through 2b; xt/yt through 2a; w_pool (w2 then M) closes
    # before 2a to make room for the att tiles.
    vo_pool = ctx.enter_context(tc.tile_pool(name="vo", bufs=1))
    es_x = ExitStack()
    xt_pool = es_x.enter_context(tc.tile_pool(name="xt", bufs=1))
    es_y = ExitStack()
    yt_pool = es_y.enter_context(tc.tile_pool(name="yt", bufs=1))
    es_w = ExitStack()
    w_pool = es_w.enter_context(tc.tile_pool(name="wp", bufs=1))

    xt = [xt_pool.tile([P, SK], BF16, tag=f"xt{i}", name=f"xt{i}") for i in range(ET)]
    yt = [yt_pool.tile([P, SQ], BF16, tag=f"yt{i}", name=f"yt{i}") for i in range(ET)]
    vo = [vo_pool.tile([P, E], BF16, tag=f"vo{i}", name=f"vo{i}") for i in range(KT)]

    # ---------------- Phase 1v: vo[s, f] = x @ W2.T (bf16, resident) ----------
    # DMA order matches consumption: w2 tiles first, then xT in column-pair
    # blocks (st-pair major) so the st-loop streams while x loads.
    w2 = []
    for et in range(ET):
        t = w_pool.tile([P, E], BF16, tag=f"w{et}", name=f"w2{et}")
        # head DMAs split across two queues: x quads on Scalar, w2 on Sync
        nc.scalar.dma_start(
            out=xt[et][:, 0:4 * P], in_=xT[et * P:(et + 1) * P, 0:4 * P])
        nc.sync.dma_start(out=t[:], in_=w2T[et * P:(et + 1) * P, :])
        w2.append(t)

    ident_f = singles.tile([P, P], F32, tag="ident_f")
    make_identity(nc, ident_f)
    ident = singles.tile([P, P], BF16, tag="ident")
    nc.vector.tensor_copy(out=ident[:], in_=ident_f[:])

    for et in range(ET):
        nc.sync.dma_start(
            out=xt[et][:, 4 * P:8 * P],
            in_=xT[et * P:(et + 1) * P, 4 * P:8 * P])
    for et in range(ET):
        nc.sync.dma_start(
            out=xt[et][:, 8 * P:16 * P],
            in_=xT[et * P:(et + 1) * P, 8 * P:16 * P])
    # M loads stream behind the x columns, consumed by phase 1y.
    wm = []
    for et in range(ET):
        t = w_pool.tile([P, E], BF16, tag=f"w{et}")
        nc.sync.dma_start(out=t[:], in_=mM[et * P:(et + 1) * P, :])
        wm.append(t)

    for st in range(0, KT, 2):
        psums = [ps_mm.tile([P, NC], F32, tag="mm", name="mmp") for _ in range(4)]
        for et in range(ET):
            for j in range(2):          # st-pair
                for fc in range(FC):
                    nc.tensor.matmul(
                        psums[2 * j + fc][:],
                        xt[et][:, (st + j) * P:(st + j + 1) * P],
                        w2[et][:, fc * NC:(fc + 1) * NC],
                        start=(et == 0), stop=(et == ET - 1),
                    )
        for j in range(2):
            for fc in range(FC):
                nc.vector.tensor_copy(
                    out=vo[st + j][:, fc * NC:(fc + 1) * NC],
                    in_=psums[2 * j + fc][:])

    # ---------------- Phase 1y: yT[f, s] = M-contract @ xq --------------------
    for ft in range(0, ET, 2):
        psums = [ps_mm.tile([P, NC], F32, tag="mm", name="mmp") for _ in range(4)]
        for et in range(ET):
            for j in range(2):          # ft-pair
                for sc in range(SQC):
                    nc.tensor.matmul(
                        psums[2 * j + sc][:],
                        wm[et][:, (ft + j) * P:(ft + j + 1) * P],
                        xt[et][:, sc * NC:(sc + 1) * NC],
                        start=(et == 0), stop=(et == ET - 1),
                    )
        for j in range(2):
            for sc in range(SQC):
                nc.vector.tensor_copy(
                    out=yt[ft + j][:, sc * NC:(sc + 1) * NC],
                    in_=psums[2 * j + sc][:])

    es_w.close()  # w2/M weights freed before the att tiles allocate

    # ---------------- Phase 2a: scores -> softmax -> attnT (bf16) -------------
    es_att = ExitStack()
    att_pool = es_att.enter_context(tc.tile_pool(name="att", bufs=1, side="right"))
    att = [att_pool.tile([P, SQ], BF16, tag=f"at{i}", name=f"at{i}") for i in range(KT)]

    with tc.tile_pool(name="exp", bufs=2) as exp_pool, \
         tc.tile_pool(name="sums", bufs=4) as sums_pool, \
         tc.tile_pool(name="ps_t", bufs=2, space="PSUM") as ps_t:

        def emit_transpose(prev_sb, prev_attn, kj):
            # transpose LDWEIGHTS hides under the surrounding 512-col matmuls
            pst = ps_t.tile([P, P], BF16, tag="pst")
            nc.tensor.transpose(
                pst[:], prev_attn[:, kj * P:(kj + 1) * P], ident[:])
            nc.vector.tensor_copy(
                out=att[kj][:, prev_sb * P:(prev_sb + 1) * P], in_=pst[:])

        pending = None  # (sb, attn_n) of the previous query block
        for sb in range(ET):  # 8 query sub-blocks of 128
            psums = [ps_mm.tile([P, NC], F32, tag="mm", name="mmp") for _ in range(SKC)]
            slot = 0
            for et in range(ET):
                for kc in range(SKC):
                    nc.tensor.matmul(
                        psums[kc][:],
                        yt[et][:, sb * P:(sb + 1) * P],
                        xt[et][:, kc * NC:(kc + 1) * NC],
                        start=(et == 0), stop=(et == ET - 1),
                    )
                    if pending is not None and slot % 2 == 1:
                        emit_transpose(pending[0], pending[1], slot // 2)
                    slot += 1
            exp_t = exp_pool.tile([P, SK], F32, tag="exp")
            sums4 = sums_pool.tile([P, SKC], F32, tag="sums4")
            for kc in range(SKC):
                nc.scalar.activation(
                    out=exp_t[:, kc * NC:(kc + 1) * NC],
                    in_=psums[kc][:], func=Exp, scale=SCALE,
                    accum_out=sums4[:, kc:kc + 1],
                )
            sum1 = sums_pool.tile([P, 1], F32, tag="sum1")
            nc.vector.tensor_reduce(
                out=sum1[:], in_=sums4[:],
                axis=mybir.AxisListType.X, op=mybir.AluOpType.add)
            recip = sums_pool.tile([P, 1], F32, tag="recip")
            nc.vector.reciprocal(out=recip[:], in_=sum1[:])
            attn_n = exp_pool.tile([P, SK], BF16, tag="attn_n")
            nc.scalar.mul(out=attn_n[:], in_=exp_t[:], mul=recip[:])
            pending = (sb, attn_n)
        for kj in range(KT):  # flush last block's transposes
            emit_transpose(pending[0], pending[1], kj)

    # ---------------- Phase 2b: outT[e, qi] = vo.T-contract @ attnT -----------
    es_y.close()  # yT freed after scores
    es_x.close()  # xT freed after scores
    aot_pool = ctx.enter_context(tc.tile_pool(name="aot", bufs=1))
    aot = [aot_pool.tile([P, SQ], BF16, tag=f"ao{i}", name=f"ao{i}") for i in range(ET)]

    # gate weights stream in during 2b
    gw_pool = ctx.enter_context(tc.tile_pool(name="gw", bufs=1))
    gw = []
    for et in range(ET):
        t = gw_pool.tile([P, E], BF16, tag=f"gw{et}", name=f"gw{et}")
        nc.sync.dma_start(out=t[:], in_=gwT[et * P:(et + 1) * P, :])
        gw.append(t)

    for et in range(0, ET, 2):
        psums = [ps_mm.tile([P, NC], F32, tag="mm", name="mmp") for _ in range(4)]
        for kj in range(KT):
            for j in range(2):          # et-pair
                for qc in range(SQC):
                    nc.tensor.matmul(
                        psums[2 * j + qc][:],
                        vo[kj][:, (et + j) * P:(et + j + 1) * P],
                        att[kj][:, qc * NC:(qc + 1) * NC],
                        start=(kj == 0), stop=(kj == KT - 1),
                    )
        for j in range(2):
            for qc in range(SQC):
                nc.vector.tensor_copy(
                    out=aot[et + j][:, qc * NC:(qc + 1) * NC],
                    in_=psums[2 * j + qc][:])

    es_att.close()  # att freed after attn@vo

    # ---------------- Phase 2c: gate, result ----------------
    with tc.tile_pool(name="fin", bufs=2) as fin_pool:
        for ft in range(0, ET, 2):
            psums = [ps_mm.tile([P, NC], F32, tag="mm", name="mmp") for _ in range(4)]
            for et in range(ET):
                for j in range(2):      # ft-pair
                    for qc in range(SQC):
                        nc.tensor.matmul(
                            psums[2 * j + qc][:],
                            gw[et][:, (ft + j) * P:(ft + j + 1) * P],
                            aot[et][:, qc * NC:(qc + 1) * NC],
                            start=(et == 0), stop=(et == ET - 1),
                        )
            for j in range(2):
                fin = fin_pool.tile([P, SQ], F32, tag=f"fin{j}")
                for qc in range(SQC):
                    gate = fin_pool.tile([P, NC], F32, tag="gate")
                    nc.scalar.activation(
                        out=gate[:], in_=psums[2 * j + qc][:], func=Sigmoid)
                    nc.vector.tensor_mul(
                        fin[:, qc * NC:(qc + 1) * NC], gate[:],
                        aot[ft + j][:, qc * NC:(qc + 1) * NC])
                nc.sync.dma_start(
                    out=outT[(ft + j) * P:(ft + j + 1) * P, :], in_=fin[:])


_NC_CACHE = None


def _get_nc():
    global _NC_CACHE
    if _NC_CACHE is None:
        _NC_CACHE = _build_nc()
    return _NC_CACHE


def _prep_in_maps(rotation_params, entangle_params, inputs, gate_w):
    w_qkv = np.asarray(rotation_params, dtype=np.float32).reshape(3 * E, E)
    wq, wk, wv = w_qkv[:E], w_qkv[E:2 * E], w_qkv[2 * E:]
    w_out = np.asarray(entangle_params, dtype=np.float32).reshape(E, E)
    gw = np.asarray(gate_w, dtype=np.float32)
    x = np.asarray(inputs, dtype=np.float32)

    import ml_dtypes
    bf16 = ml_dtypes.bfloat16
    mM = np.ascontiguousarray((wq.T @ wk).astype(bf16))    # scores = x @ M @ x.T
    w2T = np.ascontiguousarray((w_out @ wv).T.astype(bf16))  # vo = x @ (w_out@wv).T
    gwT = np.ascontiguousarray(gw.T.astype(bf16))

    in_maps = []
    for c in range(NCORES):
        b, h = c // 2, c % 2
        xT = x[b].T  # [E, S]
        if h == 1:   # rotate keys so this core's queries sit at columns 0:SQ
            xT = np.concatenate([xT[:, SQ:], xT[:, :SQ]], axis=1)
        in_maps.append({
            "xT": np.ascontiguousarray(xT.astype(bf16)),
            "mM": mM, "w2T": w2T, "gwT": gwT,
        })
    return in_maps


def _assemble(results):
    out = np.empty((B, S, E), dtype=np.float32)
    for c in range(NCORES):
        b, h = c // 2, c % 2
        out[b, h * SQ:(h + 1) * SQ, :] = results[c]["outT"].T
    return out


def _run(in_maps, trace=False):
    nc = _get_nc()
    return run_bass_kernel_spmd(nc, in_maps, core_ids=list(range(NCORES)),
                                trace=trace)


def kernel(rotation_params, entangle_params, inputs, gate_w):
    in_maps = _prep_in_maps(rotation_params, entangle_params, inputs, gate_w)
    res = _run(in_maps, trace=False)
    return _assemble(res.results)


# revision 14
# speedup vs baseline: 1.0022x; 1.0022x over previous
"""Trainium2 Bass kernel for nn_ClassicalSelfAttention (B=4, S=2048, E=1024).

Reference computation (fp32):
    w_qkv = rotation_params.reshape(3E, E); w_out = entangle_params.reshape(E, E)
    qkv = x @ w_qkv.T; q, k, v = split(qkv)
    scores = (q / sqrt(64)) @ k.T          # full-E attention, no heads
    attn = softmax(scores, axis=-1)
    out = (attn @ v) @ w_out.T
    result = sigmoid(out @ gate_w.T) * out

Host-side weight fusion (pure algebra, done once on CPU):
    scores = q @ k.T = x @ (wq.T @ wk) @ x.T          -> M  = wq.T @ wk
    (attn @ v) @ w_out.T = attn @ (x @ (w_out@wv).T)  -> W2 = w_out @ wv
so the k-projection and out-projection matmuls disappear from the device.

Sharding: 8 cores = 4 batches x 2 query-halves. Each core computes vo for its
whole batch (duplicated within the pair) and attention + gate for its 1024
queries. Key order is rotated per query-half so each core's queries are
always columns 0:1024 of its (host-pre-transposed) x^T input — softmax and
attn@vo are permutation-invariant in key order.

Precision: all matmul operands are bf16 (softmax statistics, PSUM
accumulation, and the final gate/multiply epilogue stay fp32). bf16 halves
DMA traffic, lets the whole working set stay resident in SBUF (no DRAM
spill), and its ~97ns LDWEIGHTS hides completely under 512-col matmuls so
every phase runs at the PE issue limit. Measured rel err ~7e-3 (gate 2e-2).

Layout (feature-major throughout):
    xT [e, s]  (resident through 2a)
    vo [s, f] = xT-stat @ W2T           (vo = x @ W2.T, computed first)
    yT [f, s] = M-contract @ xq         (y = x @ M)
    scores [qi, kj] = yT-stat @ xT -> softmax along free dim -> attn
    PE-transpose attn -> attnT [kj, qi]
    outT [f, qi] = vo.T-contract @ attnT
    gateT = gw-contract @ outT;  result^T = sigmoid(gateT) * outT
Host untransposes the per-core [E, 1024] result tiles.

Matmul loops interleave 4 PSUM accumulation chains (pairs of output tiles x
free-dim chunks) to hide LDWEIGHTS; each query block's 16 attn transposes
are emitted between the next block's score matmuls for the same reason.
"""

from contextlib import ExitStack

import numpy as np

import concourse.bass as bass
import concourse.tile as tile
from concourse import bacc, mybir
from concourse.bass_utils import run_bass_kernel_spmd
from concourse.masks import make_identity

F32 = mybir.dt.float32
F32R = mybir.dt.float32r
BF16 = mybir.dt.bfloat16

P = 128
E = 1024
B = 4
S = 2048
SK = S            # keys per core (full batch sequence)
SQ = S // 2       # queries per core (half)
ET = E // P       # 8 e-tiles
KT = SK // P      # 16 key tiles
NC = 512          # moving-operand chunk (f32r full speed needs >=256, max 512)
SKC = SK // NC    # 4
SQC = SQ // NC    # 2
FC = E // NC      # 2
NCORES = 8
SCALE = 1.0 / 8.0  # 1/sqrt(head_dim=64), folded into exp()


def _round_fp32r(x: np.ndarray) -> np.ndarray:
    """Round-to-nearest-even to fp32r (11-bit mantissa; low 12 bits zero)."""
    u = np.ascontiguousarray(x, dtype=np.float32).view(np.uint32).astype(np.uint64)
    r = (u + 0x7FF + ((u >> 12) & 1)) & ~np.uint64(0xFFF)
    return r.astype(np.uint32).view(np.float32)


def _build_nc():
    nc = bacc.Bacc("TRN2", target_bir_lowering=False, debug=False,
                   num_devices=NCORES)
    xT = nc.dram_tensor("xT", [E, SK], BF16, kind="ExternalInput").ap()
    mM = nc.dram_tensor("mM", [E, E], BF16, kind="ExternalInput").ap()
    w2T = nc.dram_tensor("w2T", [E, E], BF16, kind="ExternalInput").ap()
    gwT = nc.dram_tensor("gwT", [E, E], BF16, kind="ExternalInput").ap()
    outT = nc.dram_tensor("outT", [E, SQ], F32, kind="ExternalOutput").ap()

    with tile.TileContext(nc) as tc, ExitStack() as ctx:
        _emit(tc, ctx, xT, mM, w2T, gwT, outT)
    nc.compile()
    return nc


def _emit(tc, ctx, xT, mM, w2T, gwT, outT):
    nc = tc.nc
    Exp = mybir.ActivationFunctionType.Exp
    Sigmoid = mybir.ActivationFunctionType.Sigmoid

    singles = ctx.enter_context(tc.tile_pool(name="singles", bufs=1))
    ident_f = singles.tile([P, P], F32, tag="ident_f")
    make_identity(nc, ident_f)
    ident = singles.tile([P, P], BF16, tag="ident")
    nc.vector.tensor_copy(out=ident[:], in_=ident_f[:])

    ps_mm = ctx.enter_context(tc.tile_pool(name="ps_mm", bufs=6, space="PSUM"))

    # Long-lived SBUF pools, created longest-lived first (LIFO release):
    # vo lives through 2b; xt/yt through 2a; w_pool (w2 then M) closes
    # before 2a to make room for the att tiles.
    vo_pool = ctx.enter_context(tc.tile_pool(name="vo", bufs=1))
    es_x = ExitStack()
    xt_pool = es_x.enter_context(tc.tile_pool(name="xt", bufs=1))
    es_y = ExitStack()
    yt_pool = es_y.enter_context(tc.tile_pool(name="yt", bufs=1))
    es_w = ExitStack()
    w_pool = es_w.enter_context(tc.tile_pool(name="wp", bufs=1))

    xt = [xt_pool.tile([P, SK], BF16, tag=f"xt{i}", name=f"xt{i}") for i in range(ET)]
    yt = [yt_pool.tile([P, SQ], BF16, tag=f"yt{i}", name=f"yt{i}") for i in range(ET)]
    vo = [vo_pool.tile([P, E], BF16, tag=f"vo{i}", name=f"vo{i}") for i in range(KT)]

    # ---------------- Phase 1v: vo[s, f] = x @ W2.T (bf16, resident) ----------
    # DMA order matches consumption: w2 tiles first, then xT in column-pair
    # blocks (st-pair major) so the st-loop streams while x loads.
    w2 = []
    for et in range(ET):
        t = w_pool.tile([P, E], BF16, tag=f"w{et}", name=f"w2{et}")
        # head DMAs split across queues: x quads issue from the idle Scalar
        # DGE in parallel with w2 on Sync (~650ns serialized issue each)
        nc.scalar.dma_start(
            out=xt[et][:, 0:4 * P], in_=xT[et * P:(et + 1) * P, 0:4 * P])
        nc.sync.dma_start(out=t[:], in_=w2T[et * P:(et + 1) * P, :])
        w2.append(t)
    for st in range(4, KT, 4):
        for et in range(ET):
            nc.sync.dma_start(
                out=xt[et][:, st * P:(st + 4) * P],
                in_=xT[et * P:(et + 1) * P, st * P:(st + 4) * P])
    # M loads stream behind the x columns, consumed by phase 1y.
    wm = []
    for et in range(ET):
        t = w_pool.tile([P, E], BF16, tag=f"w{et}")
        nc.scalar.dma_start(out=t[:], in_=mM[et * P:(et + 1) * P, :])
        wm.append(t)

    for st in range(0, KT, 2):
        psums = [ps_mm.tile([P, NC], F32, tag="mm", name="mmp") for _ in range(4)]
        for et in range(ET):
            for j in range(2):          # st-pair
                for fc in range(FC):
                    nc.tensor.matmul(
                        psums[2 * j + fc][:],
                        xt[et][:, (st + j) * P:(st + j + 1) * P],
                        w2[et][:, fc * NC:(fc + 1) * NC],
                        start=(et == 0), stop=(et == ET - 1),
                    )
        for j in range(2):
            for fc in range(FC):
                nc.vector.tensor_copy(
                    out=vo[st + j][:, fc * NC:(fc + 1) * NC],
                    in_=psums[2 * j + fc][:])

    # ---------------- Phase 1y: yT[f, s] = M-contract @ xq --------------------
    for ft in range(0, ET, 2):
        psums = [ps_mm.tile([P, NC], F32, tag="mm", name="mmp") for _ in range(4)]
        for et in range(ET):
            for j in range(2):          # ft-pair
                for sc in range(SQC):
                    nc.tensor.matmul(
                        psums[2 * j + sc][:],
                        wm[et][:, (ft + j) * P:(ft + j + 1) * P],
                        xt[et][:, sc * NC:(sc + 1) * NC],
                        start=(et == 0), stop=(et == ET - 1),
                    )
        for j in range(2):
            for sc in range(SQC):
                nc.vector.tensor_copy(
                    out=yt[ft + j][:, sc * NC:(sc + 1) * NC],
                    in_=psums[2 * j + sc][:])

    es_w.close()  # w2/M weights freed before the att tiles allocate

    # ---------------- Phase 2a: scores -> softmax -> attnT (bf16) -------------
    es_att = ExitStack()
    att_pool = es_att.enter_context(tc.tile_pool(name="att", bufs=1, side="right"))
    att = [att_pool.tile([P, SQ], BF16, tag=f"at{i}", name=f"at{i}") for i in range(KT)]

    with tc.tile_pool(name="exp", bufs=2) as exp_pool, \
         tc.tile_pool(name="sums", bufs=4) as sums_pool, \
         tc.tile_pool(name="ps_t", bufs=2, space="PSUM") as ps_t:

        def emit_transpose(prev_sb, prev_attn, kj):
            # transpose LDWEIGHTS hides under the surrounding 512-col matmuls
            pst = ps_t.tile([P, P], BF16, tag="pst")
            nc.tensor.transpose(
                pst[:], prev_attn[:, kj * P:(kj + 1) * P], ident[:])
            nc.vector.tensor_copy(
                out=att[kj][:, prev_sb * P:(prev_sb + 1) * P], in_=pst[:])

        pending = None  # (sb, attn_n) of the previous query block
        for sb in range(ET):  # 8 query sub-blocks of 128
            psums = [ps_mm.tile([P, NC], F32, tag="mm", name="mmp") for _ in range(SKC)]
            slot = 0
            for et in range(ET):
                for kc in range(SKC):
                    nc.tensor.matmul(
                        psums[kc][:],
                        yt[et][:, sb * P:(sb + 1) * P],
                        xt[et][:, kc * NC:(kc + 1) * NC],
                        start=(et == 0), stop=(et == ET - 1),
                    )
                    if pending is not None and slot % 2 == 1:
                        emit_transpose(pending[0], pending[1], slot // 2)
                    slot += 1
            exp_t = exp_pool.tile([P, SK], F32, tag="exp")
            sums4 = sums_pool.tile([P, SKC], F32, tag="sums4")
            for kc in range(SKC):
                nc.scalar.activation(
                    out=exp_t[:, kc * NC:(kc + 1) * NC],
                    in_=psums[kc][:], func=Exp, scale=SCALE,
                    accum_out=sums4[:, kc:kc + 1],
                )
            sum1 = sums_pool.tile([P, 1], F32, tag="sum1")
            nc.vector.tensor_reduce(
                out=sum1[:], in_=sums4[:],
                axis=mybir.AxisListType.X, op=mybir.AluOpType.add)
            recip = sums_pool.tile([P, 1], F32, tag="recip")
            nc.vector.reciprocal(out=recip[:], in_=sum1[:])
            attn_n = exp_pool.tile([P, SK], BF16, tag="attn_n")
            nc.scalar.mul(out=attn_n[:], in_=exp_t[:], mul=recip[:])
            pending = (sb, attn_n)
        for kj in range(KT):  # flush last block's transposes
            emit_transpose(pending[0], pending[1], kj)

    # ---------------- Phase 2b: outT[e, qi] = vo.T-contract @ attnT -----------
    es_y.close()  # yT freed after scores
    es_x.close()  # xT freed after scores
    aot_pool = ctx.enter_context(tc.tile_pool(name="aot", bufs=1))
    aot = [aot_pool.tile([P, SQ], BF16, tag=f"ao{i}", name=f"ao{i}") for i in range(ET)]

    # gate weights stream in during 2b
    gw_pool = ctx.enter_context(tc.tile_pool(name="gw", bufs=1))
    gw = []
    for et in range(ET):
        t = gw_pool.tile([P, E], BF16, tag=f"gw{et}", name=f"gw{et}")
        nc.sync.dma_start(out=t[:], in_=gwT[et * P:(et + 1) * P, :])
        gw.append(t)

    for et in range(0, ET, 2):
        psums = [ps_mm.tile([P, NC], F32, tag="mm", name="mmp") for _ in range(4)]
        for kj in range(KT):
            for j in range(2):          # et-pair
                for qc in range(SQC):
                    nc.tensor.matmul(
                        psums[2 * j + qc][:],
                        vo[kj][:, (et + j) * P:(et + j + 1) * P],
                        att[kj][:, qc * NC:(qc + 1) * NC],
                        start=(kj == 0), stop=(kj == KT - 1),
                    )
        for j in range(2):
            for qc in range(SQC):
                nc.vector.tensor_copy(
                    out=aot[et + j][:, qc * NC:(qc + 1) * NC],
                    in_=psums[2 * j + qc][:])

    es_att.close()  # att freed after attn@vo

    # ---------------- Phase 2c: gate, result ----------------
    with tc.tile_pool(name="fin", bufs=2) as fin_pool:
        for ft in range(0, ET, 2):
            psums = [ps_mm.tile([P, NC], F32, tag="mm", name="mmp") for _ in range(4)]
            for et in range(ET):
                for j in range(2):      # ft-pair
                    for qc in range(SQC):
                        nc.tensor.matmul(
                            psums[2 * j + qc][:],
                            gw[et][:, (ft + j) * P:(ft + j + 1) * P],
                            aot[et][:, qc * NC:(qc + 1) * NC],
                            start=(et == 0), stop=(et == ET - 1),
                        )
            for j in range(2):
                fin = fin_pool.tile([P, SQ], F32, tag=f"fin{j}")
                for qc in range(SQC):
                    gate = fin_pool.tile([P, NC], F32, tag="gate")
                    nc.scalar.activation(
                        out=gate[:], in_=psums[2 * j + qc][:], func=Sigmoid)
                    nc.vector.tensor_mul(
                        fin[:, qc * NC:(qc + 1) * NC], gate[:],
                        aot[ft + j][:, qc * NC:(qc + 1) * NC])
                nc.sync.dma_start(
                    out=outT[(ft + j) * P:(ft + j + 1) * P, :], in_=fin[:])


_NC_CACHE = None


def _get_nc():
    global _NC_CACHE
    if _NC_CACHE is None:
        _NC_CACHE = _build_nc()
    return _NC_CACHE


def _prep_in_maps(rotation_params, entangle_params, inputs, gate_w):
    w_qkv = np.asarray(rotation_params, dtype=np.float32).reshape(3 * E, E)
    wq, wk, wv = w_qkv[:E], w_qkv[E:2 * E], w_qkv[2 * E:]
    w_out = np.asarray(entangle_params, dtype=np.float32).reshape(E, E)
    gw = np.asarray(gate_w, dtype=np.float32)
    x = np.asarray(inputs, dtype=np.float32)

    import ml_dtypes
    bf16 = ml_dtypes.bfloat16
    mM = np.ascontiguousarray((wq.T @ wk).astype(bf16))    # scores = x @ M @ x.T
    w2T = np.ascontiguousarray((w_out @ wv).T.astype(bf16))  # vo = x @ (w_out@wv).T
    gwT = np.ascontiguousarray(gw.T.astype(bf16))

    in_maps = []
    for c in range(NCORES):
        b, h = c // 2, c % 2
        xT = x[b].T  # [E, S]
        if h == 1:   # rotate keys so this core's queries sit at columns 0:SQ
            xT = np.concatenate([xT[:, SQ:], xT[:, :SQ]], axis=1)
        in_maps.append({
            "xT": np.ascontiguousarray(xT.astype(bf16)),
            "mM": mM, "w2T": w2T, "gwT": gwT,
        })
    return in_maps


def _assemble(results):
    out = np.empty((B, S, E), dtype=np.float32)
    for c in range(NCORES):
        b, h = c // 2, c % 2
        out[b, h * SQ:(h + 1) * SQ, :] = results[c]["outT"].T
    return out


def _run(in_maps, trace=False):
    nc = _get_nc()
    return run_bass_kernel_spmd(nc, in_maps, core_ids=list(range(NCORES)),
                                trace=trace)


def kernel(rotation_params, entangle_params, inputs, gate_w):
    in_maps = _prep_in_maps(rotation_params, entangle_params, inputs, gate_w)
    res = _run(in_maps, trace=False)
    return _assemble(res.results)


# revision 15
# speedup vs baseline: 1.0107x; 1.0085x over previous
"""Trainium2 Bass kernel for nn_ClassicalSelfAttention (B=4, S=2048, E=1024).

Reference computation (fp32):
    w_qkv = rotation_params.reshape(3E, E); w_out = entangle_params.reshape(E, E)
    qkv = x @ w_qkv.T; q, k, v = split(qkv)
    scores = (q / sqrt(64)) @ k.T          # full-E attention, no heads
    attn = softmax(scores, axis=-1)
    out = (attn @ v) @ w_out.T
    result = sigmoid(out @ gate_w.T) * out

Host-side weight fusion (pure algebra, done once on CPU):
    scores = q @ k.T = x @ (wq.T @ wk) @ x.T          -> M  = wq.T @ wk
    (attn @ v) @ w_out.T = attn @ (x @ (w_out@wv).T)  -> W2 = w_out @ wv
so the k-projection and out-projection matmuls disappear from the device.

Sharding: 8 cores = 4 batches x 2 query-halves. Each core computes vo for its
whole batch (duplicated within the pair) and attention + gate for its 1024
queries. Key order is rotated per query-half so each core's queries are
always columns 0:1024 of its (host-pre-transposed) x^T input — softmax and
attn@vo are permutation-invariant in key order.

Precision: all matmul operands are bf16 (softmax statistics, PSUM
accumulation, and the final gate/multiply epilogue stay fp32). bf16 halves
DMA traffic, lets the whole working set stay resident in SBUF (no DRAM
spill), and its ~97ns LDWEIGHTS hides completely under 512-col matmuls so
every phase runs at the PE issue limit. Measured rel err ~7e-3 (gate 2e-2).

Layout (feature-major throughout):
    xT [e, s]  (resident through 2a)
    vo [s, f] = xT-stat @ W2T           (vo = x @ W2.T, computed first)
    yT [f, s] = M-contract @ xq         (y = x @ M)
    scores [qi, kj] = yT-stat @ xT -> softmax along free dim -> attn
    PE-transpose attn -> attnT [kj, qi]
    outT [f, qi] = vo.T-contract @ attnT
    gateT = gw-contract @ outT;  result^T = sigmoid(gateT) * outT
Host untransposes the per-core [E, 1024] result tiles.

Matmul loops interleave 4 PSUM accumulation chains (pairs of output tiles x
free-dim chunks) to hide LDWEIGHTS; each query block's 16 attn transposes
are emitted between the next block's score matmuls for the same reason.
"""

from contextlib import ExitStack

import numpy as np

import concourse.bass as bass
import concourse.tile as tile
from concourse import bacc, mybir
from concourse.bass_utils import run_bass_kernel_spmd
from concourse.masks import make_identity

F32 = mybir.dt.float32
F32R = mybir.dt.float32r
BF16 = mybir.dt.bfloat16

P = 128
E = 1024
B = 4
S = 2048
SK = S            # keys per core (full batch sequence)
SQ = S // 2       # queries per core (half)
ET = E // P       # 8 e-tiles
KT = SK // P      # 16 key tiles
NC = 512          # moving-operand chunk (f32r full speed needs >=256, max 512)
SKC = SK // NC    # 4
SQC = SQ // NC    # 2
FC = E // NC      # 2
NCORES = 8
SCALE = 1.0 / 8.0  # 1/sqrt(head_dim=64), folded into exp()


def _round_fp32r(x: np.ndarray) -> np.ndarray:
    """Round-to-nearest-even to fp32r (11-bit mantissa; low 12 bits zero)."""
    u = np.ascontiguousarray(x, dtype=np.float32).view(np.uint32).astype(np.uint64)
    r = (u + 0x7FF + ((u >> 12) & 1)) & ~np.uint64(0xFFF)
    return r.astype(np.uint32).view(np.float32)


def _build_nc():
    nc = bacc.Bacc("TRN2", target_bir_lowering=False, debug=False,
                   num_devices=NCORES)
    xT = nc.dram_tensor("xT", [E, SK], BF16, kind="ExternalInput").ap()
    mM = nc.dram_tensor("mM", [E, E], BF16, kind="ExternalInput").ap()
    w2T = nc.dram_tensor("w2T", [E, E], BF16, kind="ExternalInput").ap()
    gwT = nc.dram_tensor("gwT", [E, E], BF16, kind="ExternalInput").ap()
    outT = nc.dram_tensor("outT", [E, SQ], F32, kind="ExternalOutput").ap()

    with tile.TileContext(nc) as tc, ExitStack() as ctx:
        _emit(tc, ctx, xT, mM, w2T, gwT, outT)
    nc.compile()
    return nc


def _emit(tc, ctx, xT, mM, w2T, gwT, outT):
    nc = tc.nc
    Exp = mybir.ActivationFunctionType.Exp
    Sigmoid = mybir.ActivationFunctionType.Sigmoid

    singles = ctx.enter_context(tc.tile_pool(name="singles", bufs=1))
    ident_f = singles.tile([P, P], F32, tag="ident_f")
    make_identity(nc, ident_f)
    ident = singles.tile([P, P], BF16, tag="ident")
    nc.vector.tensor_copy(out=ident[:], in_=ident_f[:])

    ps_mm = ctx.enter_context(tc.tile_pool(name="ps_mm", bufs=6, space="PSUM"))

    # Long-lived SBUF pools, created longest-lived first (LIFO release):
    # vo lives through 2b; xt/yt through 2a; w_pool (w2 then M) closes
    # before 2a to make room for the att tiles.
    vo_pool = ctx.enter_context(tc.tile_pool(name="vo", bufs=1))
    es_x = ExitStack()
    xt_pool = es_x.enter_context(tc.tile_pool(name="xt", bufs=1))
    es_y = ExitStack()
    yt_pool = es_y.enter_context(tc.tile_pool(name="yt", bufs=1))
    es_w = ExitStack()
    w_pool = es_w.enter_context(tc.tile_pool(name="wp", bufs=1))

    xt = [xt_pool.tile([P, SK], BF16, tag=f"xt{i}", name=f"xt{i}") for i in range(ET)]
    yt = [yt_pool.tile([P, SQ], BF16, tag=f"yt{i}", name=f"yt{i}") for i in range(ET)]
    vo = [vo_pool.tile([P, E], BF16, tag=f"vo{i}", name=f"vo{i}") for i in range(KT)]

    # ---------------- Phase 1v: vo[s, f] = x @ W2.T (bf16, resident) ----------
    # DMA order matches consumption: w2 tiles first, then xT in column-pair
    # blocks (st-pair major) so the st-loop streams while x loads.
    w2 = []
    for et in range(ET):
        t = w_pool.tile([P, E], BF16, tag=f"w{et}", name=f"w2{et}")
        nc.sync.dma_start(
            out=xt[et][:, 0:4 * P], in_=xT[et * P:(et + 1) * P, 0:4 * P])
        nc.sync.dma_start(out=t[:], in_=w2T[et * P:(et + 1) * P, :])
        w2.append(t)
    for st in range(4, KT, 4):
        for et in range(ET):
            nc.sync.dma_start(
                out=xt[et][:, st * P:(st + 4) * P],
                in_=xT[et * P:(et + 1) * P, st * P:(st + 4) * P])
    # M loads stream behind the x columns, consumed by phase 1y.
    wm = []
    for et in range(ET):
        t = w_pool.tile([P, E], BF16, tag=f"w{et}")
        nc.sync.dma_start(out=t[:], in_=mM[et * P:(et + 1) * P, :])
        wm.append(t)

    for st in range(0, KT, 2):
        psums = [ps_mm.tile([P, NC], F32, tag="mm", name="mmp") for _ in range(4)]
        for et in range(ET):
            for j in range(2):          # st-pair
                for fc in range(FC):
                    nc.tensor.matmul(
                        psums[2 * j + fc][:],
                        xt[et][:, (st + j) * P:(st + j + 1) * P],
                        w2[et][:, fc * NC:(fc + 1) * NC],
                        start=(et == 0), stop=(et == ET - 1),
                    )
        for j in range(2):
            for fc in range(FC):
                nc.vector.tensor_copy(
                    out=vo[st + j][:, fc * NC:(fc + 1) * NC],
                    in_=psums[2 * j + fc][:])

    # ---------------- Phase 1y: yT[f, s] = M-contract @ xq --------------------
    for ft in range(0, ET, 2):
        psums = [ps_mm.tile([P, NC], F32, tag="mm", name="mmp") for _ in range(4)]
        for et in range(ET):
            for j in range(2):          # ft-pair
                for sc in range(SQC):
                    nc.tensor.matmul(
                        psums[2 * j + sc][:],
                        wm[et][:, (ft + j) * P:(ft + j + 1) * P],
                        xt[et][:, sc * NC:(sc + 1) * NC],
                        start=(et == 0), stop=(et == ET - 1),
                    )
        for j in range(2):
            for sc in range(SQC):
                nc.vector.tensor_copy(
                    out=yt[ft + j][:, sc * NC:(sc + 1) * NC],
                    in_=psums[2 * j + sc][:])

    es_w.close()  # w2/M weights freed before the att tiles allocate

    # ---------------- Phase 2a: scores -> softmax -> attnT (bf16) -------------
    es_att = ExitStack()
    att_pool = es_att.enter_context(tc.tile_pool(name="att", bufs=1, side="right"))
    att = [att_pool.tile([P, SQ], BF16, tag=f"at{i}", name=f"at{i}") for i in range(KT)]

    with tc.tile_pool(name="exp", bufs=2) as exp_pool, \
         tc.tile_pool(name="sums", bufs=4) as sums_pool, \
         tc.tile_pool(name="ps_t", bufs=2, space="PSUM") as ps_t:

        def emit_transpose(prev_sb, prev_attn, kj):
            # transpose LDWEIGHTS hides under the surrounding 512-col matmuls
            pst = ps_t.tile([P, P], BF16, tag="pst")
            nc.tensor.transpose(
                pst[:], prev_attn[:, kj * P:(kj + 1) * P], ident[:])
            nc.vector.tensor_copy(
                out=att[kj][:, prev_sb * P:(prev_sb + 1) * P], in_=pst[:])

        pending = None  # (sb, attn_n) of the previous query block
        for sb in range(ET):  # 8 query sub-blocks of 128
            psums = [ps_mm.tile([P, NC], F32, tag="mm", name="mmp") for _ in range(SKC)]
            slot = 0
            for et in range(ET):
                for kc in range(SKC):
                    nc.tensor.matmul(
                        psums[kc][:],
                        yt[et][:, sb * P:(sb + 1) * P],
                        xt[et][:, kc * NC:(kc + 1) * NC],
                        start=(et == 0), stop=(et == ET - 1),
                    )
                    if pending is not None and slot % 2 == 1:
                        emit_transpose(pending[0], pending[1], slot // 2)
                    slot += 1
            exp_t = exp_pool.tile([P, SK], F32, tag="exp")
            sums4 = sums_pool.tile([P, SKC], F32, tag="sums4")
            for kc in range(SKC):
                nc.scalar.activation(
                    out=exp_t[:, kc * NC:(kc + 1) * NC],
                    in_=psums[kc][:], func=Exp, scale=SCALE,
                    accum_out=sums4[:, kc:kc + 1],
                )
            sum1 = sums_pool.tile([P, 1], F32, tag="sum1")
            nc.vector.tensor_reduce(
                out=sum1[:], in_=sums4[:],
                axis=mybir.AxisListType.X, op=mybir.AluOpType.add)
            recip = sums_pool.tile([P, 1], F32, tag="recip")
            nc.vector.reciprocal(out=recip[:], in_=sum1[:])
            attn_n = exp_pool.tile([P, SK], BF16, tag="attn_n")
            nc.scalar.mul(out=attn_n[:], in_=exp_t[:], mul=recip[:])
            pending = (sb, attn_n)
        for kj in range(KT):  # flush last block's transposes
            emit_transpose(pending[0], pending[1], kj)

    # ---------------- Phase 2b: outT[e, qi] = vo.T-contract @ attnT -----------
    es_y.close()  # yT freed after scores
    es_x.close()  # xT freed after scores
    aot_pool = ctx.enter_context(tc.tile_pool(name="aot", bufs=1))
    aot = [aot_pool.tile([P, SQ], BF16, tag=f"ao{i}", name=f"ao{i}") for i in range(ET)]

    # gate weights stream in during 2b
    gw_pool = ctx.enter_context(tc.tile_pool(name="gw", bufs=1))
    gw = []
    for et in range(ET):
        t = gw_pool.tile([P, E], BF16, tag=f"gw{et}", name=f"gw{et}")
        nc.sync.dma_start(out=t[:], in_=gwT[et * P:(et + 1) * P, :])
        gw.append(t)

    for et in range(0, ET, 2):
        psums = [ps_mm.tile([P, NC], F32, tag="mm", name="mmp") for _ in range(4)]
        for kj in range(KT):
            for j in range(2):          # et-pair
                for qc in range(SQC):
                    nc.tensor.matmul(
                        psums[2 * j + qc][:],
                        vo[kj][:, (et + j) * P:(et + j + 1) * P],
                        att[kj][:, qc * NC:(qc + 1) * NC],
                        start=(kj == 0), stop=(kj == KT - 1),
                    )
        for j in range(2):
            for qc in range(SQC):
                nc.vector.tensor_copy(
                    out=aot[et + j][:, qc * NC:(qc + 1) * NC],
                    in_=psums[2 * j + qc][:])

    es_att.close()  # att freed after attn@vo

    # ---------------- Phase 2c: gate, result ----------------
    with tc.tile_pool(name="fin", bufs=2) as fin_pool:
        for ft in range(0, ET, 2):
            psums = [ps_mm.tile([P, NC], F32, tag="mm", name="mmp") for _ in range(4)]
            for et in range(ET):
                for j in range(2):      # ft-pair
                    for qc in range(SQC):
                        nc.tensor.matmul(
                            psums[2 * j + qc][:],
                            gw[et][:, (ft + j) * P:(ft + j + 1) * P],
                            aot[et][:, qc * NC:(qc + 1) * NC],
                            start=(et == 0), stop=(et == ET - 1),
                        )
            for j in range(2):
                fin = fin_pool.tile([P, SQ], F32, tag=f"fin{j}")
                for qc in range(SQC):
                    gate = fin_pool.tile([P, NC], F32, tag="gate")
                    nc.scalar.activation(
                        out=gate[:], in_=psums[2 * j + qc][:], func=Sigmoid)
                    nc.vector.tensor_mul(
                        fin[:, qc * NC:(qc + 1) * NC], gate[:],
                        aot[ft + j][:, qc * NC:(qc + 1) * NC])
                nc.sync.dma_start(
                    out=outT[(ft + j) * P:(ft + j + 1) * P, :], in_=fin[:])


_NC_CACHE = None


def _get_nc():
    global _NC_CACHE
    if _NC_CACHE is None:
        _NC_CACHE = _build_nc()
    return _NC_CACHE


def _prep_in_maps(rotation_params, entangle_params, inputs, gate_w):
    w_qkv = np.asarray(rotation_params, dtype=np.float32).reshape(3 * E, E)
    wq, wk, wv = w_qkv[:E], w_qkv[E:2 * E], w_qkv[2 * E:]
    w_out = np.asarray(entangle_params, dtype=np.float32).reshape(E, E)
    gw = np.asarray(gate_w, dtype=np.float32)
    x = np.asarray(inputs, dtype=np.float32)

    import ml_dtypes
    bf16 = ml_dtypes.bfloat16
    mM = np.ascontiguousarray((wq.T @ wk).astype(bf16))    # scores = x @ M @ x.T
    w2T = np.ascontiguousarray((w_out @ wv).T.astype(bf16))  # vo = x @ (w_out@wv).T
    gwT = np.ascontiguousarray(gw.T.astype(bf16))

    in_maps = []
    for c in range(NCORES):
        b, h = c // 2, c % 2
        xT = x[b].T  # [E, S]
        if h == 1:   # rotate keys so this core's queries sit at columns 0:SQ
            xT = np.concatenate([xT[:, SQ:], xT[:, :SQ]], axis=1)
        in_maps.append({
            "xT": np.ascontiguousarray(xT.astype(bf16)),
            "mM": mM, "w2T": w2T, "gwT": gwT,
        })
    return in_maps


def _assemble(results):
    out = np.empty((B, S, E), dtype=np.float32)
    for c in range(NCORES):
        b, h = c // 2, c % 2
        out[b, h * SQ:(h + 1) * SQ, :] = results[c]["outT"].T
    return out


def _run(in_maps, trace=False):
    nc = _get_nc()
    return run_bass_kernel_spmd(nc, in_maps, core_ids=list(range(NCORES)),
                                trace=trace)


def kernel(rotation_params, entangle_params, inputs, gate_w):
    in_maps = _prep_in_maps(rotation_params, entangle_params, inputs, gate_w)
    res = _run(in_maps, trace=False)
    return _assemble(res.results)


# revision 16
# speedup vs baseline: 1.0128x; 1.0021x over previous
"""Trainium2 Bass kernel for nn_ClassicalSelfAttention (B=4, S=2048, E=1024).

Reference computation (fp32):
    w_qkv = rotation_params.reshape(3E, E); w_out = entangle_params.reshape(E, E)
    qkv = x @ w_qkv.T; q, k, v = split(qkv)
    scores = (q / sqrt(64)) @ k.T          # full-E attention, no heads
    attn = softmax(scores, axis=-1)
    out = (attn @ v) @ w_out.T
    result = sigmoid(out @ gate_w.T) * out

Host-side weight fusion (pure algebra, done once on CPU):
    scores = q @ k.T = x @ (wq.T @ wk) @ x.T          -> M  = wq.T @ wk
    (attn @ v) @ w_out.T = attn @ (x @ (w_out@wv).T)  -> W2 = w_out @ wv
so the k-projection and out-projection matmuls disappear from the device.

Sharding: 8 cores = 4 batches x 2 query-halves. Each core computes vo for its
whole batch (duplicated within the pair) and attention + gate for its 1024
queries. Key order is rotated per query-half so each core's queries are
always columns 0:1024 of its (host-pre-transposed) x^T input — softmax and
attn@vo are permutation-invariant in key order.

Precision: all matmul operands are bf16 (softmax statistics, PSUM
accumulation, and the final gate/multiply epilogue stay fp32). bf16 halves
DMA traffic, lets the whole working set stay resident in SBUF (no DRAM
spill), and its ~97ns LDWEIGHTS hides completely under 512-col matmuls so
every phase runs at the PE issue limit. Measured rel err ~7e-3 (gate 2e-2).

Layout (feature-major throughout):
    xT [e, s]  (resident through 2a)
    vo [s, f] = xT-stat @ W2T           (vo = x @ W2.T, computed first)
    yT [f, s] = M-contract @ xq         (y = x @ M)
    scores [qi, kj] = yT-stat @ xT -> softmax along free dim -> attn
    PE-transpose attn -> attnT [kj, qi]
    outT [f, qi] = vo.T-contract @ attnT
    gateT = gw-contract @ outT;  result^T = sigmoid(gateT) * outT
Host untransposes the per-core [E, 1024] result tiles.

Matmul loops interleave 4 PSUM accumulation chains (pairs of output tiles x
free-dim chunks) to hide LDWEIGHTS; each query block's 16 attn transposes
are emitted between the next block's score matmuls for the same reason.
"""

from contextlib import ExitStack

import numpy as np

import concourse.bass as bass
import concourse.tile as tile
from concourse import bacc, mybir
from concourse.bass_utils import run_bass_kernel_spmd
from concourse.masks import make_identity

F32 = mybir.dt.float32
F32R = mybir.dt.float32r
BF16 = mybir.dt.bfloat16

P = 128
E = 1024
B = 4
S = 2048
SK = S            # keys per core (full batch sequence)
SQ = S // 2       # queries per core (half)
ET = E // P       # 8 e-tiles
KT = SK // P      # 16 key tiles
NC = 512          # moving-operand chunk (f32r full speed needs >=256, max 512)
SKC = SK // NC    # 4
SQC = SQ // NC    # 2
FC = E // NC      # 2
NCORES = 8
SCALE = 1.0 / 8.0  # 1/sqrt(head_dim=64), folded into exp()


def _round_fp32r(x: np.ndarray) -> np.ndarray:
    """Round-to-nearest-even to fp32r (11-bit mantissa; low 12 bits zero)."""
    u = np.ascontiguousarray(x, dtype=np.float32).view(np.uint32).astype(np.uint64)
    r = (u + 0x7FF + ((u >> 12) & 1)) & ~np.uint64(0xFFF)
    return r.astype(np.uint32).view(np.float32)


def _build_nc():
    nc = bacc.Bacc("TRN2", target_bir_lowering=False, debug=False,
                   num_devices=NCORES)
    xT = nc.dram_tensor("xT", [E, SK], BF16, kind="ExternalInput").ap()
    mM = nc.dram_tensor("mM", [E, E], BF16, kind="ExternalInput").ap()
    w2T = nc.dram_tensor("w2T", [E, E], BF16, kind="ExternalInput").ap()
    gwT = nc.dram_tensor("gwT", [E, E], BF16, kind="ExternalInput").ap()
    outT = nc.dram_tensor("outT", [E, SQ], F32, kind="ExternalOutput").ap()

    with tile.TileContext(nc) as tc, ExitStack() as ctx:
        _emit(tc, ctx, xT, mM, w2T, gwT, outT)
    nc.compile()
    return nc


def _emit(tc, ctx, xT, mM, w2T, gwT, outT):
    nc = tc.nc
    Exp = mybir.ActivationFunctionType.Exp
    Sigmoid = mybir.ActivationFunctionType.Sigmoid

    singles = ctx.enter_context(tc.tile_pool(name="singles", bufs=1))
    ident_f = singles.tile([P, P], F32, tag="ident_f")
    make_identity(nc, ident_f)
    ident = singles.tile([P, P], BF16, tag="ident")
    nc.vector.tensor_copy(out=ident[:], in_=ident_f[:])

    ps_mm = ctx.enter_context(tc.tile_pool(name="ps_mm", bufs=6, space="PSUM"))

    # Long-lived SBUF pools, created longest-lived first (LIFO release):
    # vo lives through 2b; xt/yt through 2a; w_pool (w2 then M) closes
    # before 2a to make room for the att tiles.
    vo_pool = ctx.enter_context(tc.tile_pool(name="vo", bufs=1))
    es_x = ExitStack()
    xt_pool = es_x.enter_context(tc.tile_pool(name="xt", bufs=1))
    es_y = ExitStack()
    yt_pool = es_y.enter_context(tc.tile_pool(name="yt", bufs=1))
    es_w = ExitStack()
    w_pool = es_w.enter_context(tc.tile_pool(name="wp", bufs=1))

    # x split into one tile per DMA so consumers wait only on the columns
    # they actually read (dependencies are tile-granular)
    xa = [xt_pool.tile([P, 4 * P], BF16, tag=f"xa{i}", name=f"xa{i}") for i in range(ET)]
    xb = [xt_pool.tile([P, 4 * P], BF16, tag=f"xb{i}", name=f"xb{i}") for i in range(ET)]
    xc = [xt_pool.tile([P, 8 * P], BF16, tag=f"xc{i}", name=f"xc{i}") for i in range(ET)]

    def xstat(et, st):  # [P,128] stationary column block st of x^T
        if st < 4:
            return xa[et][:, st * P:(st + 1) * P]
        if st < 8:
            return xb[et][:, (st - 4) * P:(st - 3) * P]
        return xc[et][:, (st - 8) * P:(st - 7) * P]

    def xmov(et, kc):  # [P,512] moving chunk kc of x^T
        if kc == 0:
            return xa[et][:, 0:NC]
        if kc == 1:
            return xb[et][:, 0:NC]
        return xc[et][:, (kc - 2) * NC:(kc - 1) * NC]
    yt = [yt_pool.tile([P, SQ], BF16, tag=f"yt{i}", name=f"yt{i}") for i in range(ET)]
    vo = [vo_pool.tile([P, E], BF16, tag=f"vo{i}", name=f"vo{i}") for i in range(KT)]

    # ---------------- Phase 1v: vo[s, f] = x @ W2.T (bf16, resident) ----------
    # DMA order matches consumption: w2 tiles first, then xT in column-pair
    # blocks (st-pair major) so the st-loop streams while x loads.
    w2 = []
    for et in range(ET):
        t = w_pool.tile([P, E], BF16, tag=f"w{et}", name=f"w2{et}")
        nc.sync.dma_start(
            out=xa[et][:], in_=xT[et * P:(et + 1) * P, 0:4 * P])
        nc.sync.dma_start(out=t[:], in_=w2T[et * P:(et + 1) * P, :])
        w2.append(t)
    for et in range(ET):
        nc.sync.dma_start(
            out=xb[et][:], in_=xT[et * P:(et + 1) * P, 4 * P:8 * P])
    for et in range(ET):
        nc.sync.dma_start(
            out=xc[et][:], in_=xT[et * P:(et + 1) * P, 8 * P:16 * P])
    # M loads stream behind the x columns, consumed by phase 1y.
    wm = []
    for et in range(ET):
        t = w_pool.tile([P, E], BF16, tag=f"w{et}")
        nc.sync.dma_start(out=t[:], in_=mM[et * P:(et + 1) * P, :])
        wm.append(t)

    for st in range(0, KT, 2):
        psums = [ps_mm.tile([P, NC], F32, tag="mm", name="mmp") for _ in range(4)]
        for et in range(ET):
            for j in range(2):          # st-pair
                for fc in range(FC):
                    nc.tensor.matmul(
                        psums[2 * j + fc][:],
                        xstat(et, st + j),
                        w2[et][:, fc * NC:(fc + 1) * NC],
                        start=(et == 0), stop=(et == ET - 1),
                    )
        for j in range(2):
            for fc in range(FC):
                nc.vector.tensor_copy(
                    out=vo[st + j][:, fc * NC:(fc + 1) * NC],
                    in_=psums[2 * j + fc][:])

    # ---------------- Phase 1y: yT[f, s] = M-contract @ xq --------------------
    for ft in range(0, ET, 2):
        psums = [ps_mm.tile([P, NC], F32, tag="mm", name="mmp") for _ in range(4)]
        for et in range(ET):
            for j in range(2):          # ft-pair
                for sc in range(SQC):
                    nc.tensor.matmul(
                        psums[2 * j + sc][:],
                        wm[et][:, (ft + j) * P:(ft + j + 1) * P],
                        xmov(et, sc),
                        start=(et == 0), stop=(et == ET - 1),
                    )
        for j in range(2):
            for sc in range(SQC):
                nc.vector.tensor_copy(
                    out=yt[ft + j][:, sc * NC:(sc + 1) * NC],
                    in_=psums[2 * j + sc][:])

    es_w.close()  # w2/M weights freed before the att tiles allocate

    # ---------------- Phase 2a: scores -> softmax -> attnT (bf16) -------------
    es_att = ExitStack()
    att_pool = es_att.enter_context(tc.tile_pool(name="att", bufs=1, side="right"))
    att = [att_pool.tile([P, SQ], BF16, tag=f"at{i}", name=f"at{i}") for i in range(KT)]

    with tc.tile_pool(name="exp", bufs=2) as exp_pool, \
         tc.tile_pool(name="sums", bufs=4) as sums_pool, \
         tc.tile_pool(name="ps_t", bufs=2, space="PSUM") as ps_t:

        def emit_transpose(prev_sb, prev_attn, kj):
            # transpose LDWEIGHTS hides under the surrounding 512-col matmuls
            pst = ps_t.tile([P, P], BF16, tag="pst")
            nc.tensor.transpose(
                pst[:], prev_attn[:, kj * P:(kj + 1) * P], ident[:])
            nc.vector.tensor_copy(
                out=att[kj][:, prev_sb * P:(prev_sb + 1) * P], in_=pst[:])

        pending = None  # (sb, attn_n) of the previous query block
        for sb in range(ET):  # 8 query sub-blocks of 128
            psums = [ps_mm.tile([P, NC], F32, tag="mm", name="mmp") for _ in range(SKC)]
            slot = 0
            for et in range(ET):
                for kc in range(SKC):
                    nc.tensor.matmul(
                        psums[kc][:],
                        yt[et][:, sb * P:(sb + 1) * P],
                        xmov(et, kc),
                        start=(et == 0), stop=(et == ET - 1),
                    )
                    if pending is not None and slot % 2 == 1:
                        emit_transpose(pending[0], pending[1], slot // 2)
                    slot += 1
            exp_t = exp_pool.tile([P, SK], F32, tag="exp")
            sums4 = sums_pool.tile([P, SKC], F32, tag="sums4")
            for kc in range(SKC):
                nc.scalar.activation(
                    out=exp_t[:, kc * NC:(kc + 1) * NC],
                    in_=psums[kc][:], func=Exp, scale=SCALE,
                    accum_out=sums4[:, kc:kc + 1],
                )
            sum1 = sums_pool.tile([P, 1], F32, tag="sum1")
            nc.vector.tensor_reduce(
                out=sum1[:], in_=sums4[:],
                axis=mybir.AxisListType.X, op=mybir.AluOpType.add)
            recip = sums_pool.tile([P, 1], F32, tag="recip")
            nc.vector.reciprocal(out=recip[:], in_=sum1[:])
            attn_n = exp_pool.tile([P, SK], BF16, tag="attn_n")
            nc.scalar.mul(out=attn_n[:], in_=exp_t[:], mul=recip[:])
            pending = (sb, attn_n)
        for kj in range(KT):  # flush last block's transposes
            emit_transpose(pending[0], pending[1], kj)

    # ---------------- Phase 2b: outT[e, qi] = vo.T-contract @ attnT -----------
    es_y.close()  # yT freed after scores
    es_x.close()  # xT freed after scores
    aot_pool = ctx.enter_context(tc.tile_pool(name="aot", bufs=1))
    aot = [aot_pool.tile([P, SQ], BF16, tag=f"ao{i}", name=f"ao{i}") for i in range(ET)]

    # gate weights stream in during 2b
    gw_pool = ctx.enter_context(tc.tile_pool(name="gw", bufs=1))
    gw = []
    for et in range(ET):
        t = gw_pool.tile([P, E], BF16, tag=f"gw{et}", name=f"gw{et}")
        nc.sync.dma_start(out=t[:], in_=gwT[et * P:(et + 1) * P, :])
        gw.append(t)

    for et in range(0, ET, 2):
        psums = [ps_mm.tile([P, NC], F32, tag="mm", name="mmp") for _ in range(4)]
        for kj in range(KT):
            for j in range(2):          # et-pair
                for qc in range(SQC):
                    nc.tensor.matmul(
                        psums[2 * j + qc][:],
                        vo[kj][:, (et + j) * P:(et + j + 1) * P],
                        att[kj][:, qc * NC:(qc + 1) * NC],
                        start=(kj == 0), stop=(kj == KT - 1),
                    )
        for j in range(2):
            for qc in range(SQC):
                nc.vector.tensor_copy(
                    out=aot[et + j][:, qc * NC:(qc + 1) * NC],
                    in_=psums[2 * j + qc][:])

    es_att.close()  # att freed after attn@vo

    # ---------------- Phase 2c: gate, result ----------------
    with tc.tile_pool(name="fin", bufs=2) as fin_pool:
        for ft in range(0, ET, 2):
            psums = [ps_mm.tile([P, NC], F32, tag="mm", name="mmp") for _ in range(4)]
            for et in range(ET):
                for j in range(2):      # ft-pair
                    for qc in range(SQC):
                        nc.tensor.matmul(
                            psums[2 * j + qc][:],
                            gw[et][:, (ft + j) * P:(ft + j + 1) * P],
                            aot[et][:, qc * NC:(qc + 1) * NC],
                            start=(et == 0), stop=(et == ET - 1),
                        )
            for j in range(2):
                fin = fin_pool.tile([P, SQ], F32, tag=f"fin{j}")
                for qc in range(SQC):
                    gate = fin_pool.tile([P, NC], F32, tag="gate")
                    nc.scalar.activation(
                        out=gate[:], in_=psums[2 * j + qc][:], func=Sigmoid)
                    nc.vector.tensor_mul(
                        fin[:, qc * NC:(qc + 1) * NC], gate[:],
                        aot[ft + j][:, qc * NC:(qc + 1) * NC])
                nc.sync.dma_start(
                    out=outT[(ft + j) * P:(ft + j + 1) * P, :], in_=fin[:])


_NC_CACHE = None


def _get_nc():
    global _NC_CACHE
    if _NC_CACHE is None:
        _NC_CACHE = _build_nc()
    return _NC_CACHE


def _prep_in_maps(rotation_params, entangle_params, inputs, gate_w):
    w_qkv = np.asarray(rotation_params, dtype=np.float32).reshape(3 * E, E)
    wq, wk, wv = w_qkv[:E], w_qkv[E:2 * E], w_qkv[2 * E:]
    w_out = np.asarray(entangle_params, dtype=np.float32).reshape(E, E)
    gw = np.asarray(gate_w, dtype=np.float32)
    x = np.asarray(inputs, dtype=np.float32)

    import ml_dtypes
    bf16 = ml_dtypes.bfloat16
    mM = np.ascontiguousarray((wq.T @ wk).astype(bf16))    # scores = x @ M @ x.T
    w2T = np.ascontiguousarray((w_out @ wv).T.astype(bf16))  # vo = x @ (w_out@wv).T
    gwT = np.ascontiguousarray(gw.T.astype(bf16))

    in_maps = []
    for c in range(NCORES):
        b, h = c // 2, c % 2
        xT = x[b].T  # [E, S]
        if h == 1:   # rotate keys so this core's queries sit at columns 0:SQ
            xT = np.concatenate([xT[:, SQ:], xT[:, :SQ]], axis=1)
        in_maps.append({
            "xT": np.ascontiguousarray(xT.astype(bf16)),
            "mM": mM, "w2T": w2T, "gwT": gwT,
        })
    return in_maps


def _assemble(results):
    out = np.empty((B, S, E), dtype=np.float32)
    for c in range(NCORES):
        b, h = c // 2, c % 2
        out[b, h * SQ:(h + 1) * SQ, :] = results[c]["outT"].T
    return out


def _run(in_maps, trace=False):
    nc = _get_nc()
    return run_bass_kernel_spmd(nc, in_maps, core_ids=list(range(NCORES)),
                                trace=trace)


def kernel(rotation_params, entangle_params, inputs, gate_w):
    in_maps = _prep_in_maps(rotation_params, entangle_params, inputs, gate_w)
    res = _run(in_maps, trace=False)
    return _assemble(res.results)
